# revision 50
# baseline (speedup 1.0000x reference)
"""Trainium2 Bass kernel for nn_MixtureOfAttentionHeads.

Sharding: 8 cores = 4 batches x 2 heads. Core c handles batch c//2, head c%2:
it computes all three attention types (global/rel/local) for its head over the
full sequence, applies the router gate per token, then a per-chunk pairwise
ReduceScatter sums the two heads (0.5 factor folded into Wv) and splits each
512-token chunk across the pair; each core projects its quarter-chunks with
W_o. The four chunk collectives are pipelined against later-chunk compute;
chunk k's RS output is consumed (W_o + store) while chunk k+1 computes.

Q/K projections run as fp8(e4m3) DoubleRow matmuls (weights prescaled x32,
unscaled in the exp), which quarters their PE cost; V/router projection stays
f32r for router-logit fidelity (top-k flips dominate the error budget).

The SPMD program is identical on every core; all per-core differences
(batch, head weights, rel-bias tiles) arrive as input data.
"""

import os
import sys

sys.path.insert(0, "/opt/trn_rl_repo")

import numpy as np
import ml_dtypes

# --- problem constants (hardcoded per contract) ---
B, T, D, DH = 4, 2048, 1024, 128
H, NT, TOPK = 2, 3, 2
WIN, MRP = 128, 32
SCALE = float(np.sqrt(DH))
NB = T // 128          # 16 query/key blocks
NCH = T // 512         # 4 query chunks of 512
BIG = 1.0e9
WS = 32.0              # fp8 weight prescale (q,k each carry x32)
QKS = 1.0 / (WS * WS * SCALE)   # exp scale undoing both prescales
MSC = WS * WS * SCALE  # host-side premultiplier for additive score masks

_CACHE = {}


def _build_nc():
    import concourse.bass as bass  # noqa: F401
    import concourse.mybir as mybir
    import concourse.tile as tile
    from concourse import bacc
    from concourse.masks import make_identity
    from contextlib import ExitStack

    dt = mybir.dt
    f32, f32r, bf16, f8 = dt.float32, dt.float32r, dt.bfloat16, dt.float8e4
    AX = mybir.AxisListType
    ALU = mybir.AluOpType
    ACTF = mybir.ActivationFunctionType
    DR = mybir.MatmulPerfMode.DoubleRow

    nc = bacc.Bacc("TRN2", target_bir_lowering=False, num_devices=8)

    NV = NT * DH + NT + 1  # 388 (even, fp32r requirement)

    xT = nc.dram_tensor("xT", [D, T], f32r, kind="ExternalInput")
    x8 = nc.dram_tensor("x8", [D, T], f8, kind="ExternalInput")
    wq8 = nc.dram_tensor("wq8", [128, NT, 4, 2, 128], f8, kind="ExternalInput")
    wk8 = nc.dram_tensor("wk8", [128, NT, 4, 2, 128], f8, kind="ExternalInput")
    wv = nc.dram_tensor("wv", [D, NV], f32r, kind="ExternalInput")
    masks = nc.dram_tensor("masks", [NT, 2, 128, 128], bf16, kind="ExternalInput")
    biasv = nc.dram_tensor("biasv", [128, NT], f32, kind="ExternalInput")
    bb = nc.dram_tensor("bb", [128, NT], f32, kind="ExternalInput")
    Wo = nc.dram_tensor("Wo", [DH, D], bf16, kind="ExternalInput")
    wob = nc.dram_tensor("wob", [128, D], f32, kind="ExternalInput")
    out = nc.dram_tensor("out", [T // 2, D], f32, kind="ExternalOutput")

    with tile.TileContext(nc) as tc, ExitStack() as ctx:
        persist = ctx.enter_context(tc.tile_pool(name="persist", bufs=1))
        xsp = ctx.enter_context(tc.tile_pool(name="xsp", bufs=16))
        x8p = ctx.enter_context(tc.tile_pool(name="x8p", bufs=8))
        gatep = ctx.enter_context(tc.tile_pool(name="gatep", bufs=8))
        expp = ctx.enter_context(tc.tile_pool(name="expp", bufs=18))
        avs = ctx.enter_context(tc.tile_pool(name="avs", bufs=8))
        combp = ctx.enter_context(tc.tile_pool(name="combp", bufs=4))
        finp = ctx.enter_context(tc.tile_pool(name="finp", bufs=4))
        dram = ctx.enter_context(tc.tile_pool(name="dram", bufs=1, space="DRAM"))
        php = ctx.enter_context(tc.tile_pool(name="php", bufs=2, space="PSUM"))
        sp = ctx.enter_context(tc.tile_pool(name="sp", bufs=2, space="PSUM"))
        avp = ctx.enter_context(tc.tile_pool(name="avp", bufs=2, space="PSUM"))

        # ---- persistent SBUF tensors ----
        qT = [persist.tile([128, T], bf16, tag=f"qT{t}", name=f"qT{t}") for t in range(NT)]
        kT = [persist.tile([128, T], bf16, tag=f"kT{t}", name=f"kT{t}") for t in range(NT)]
        V3 = persist.tile([128, NB, NT, DH + 2], bf16, tag="V3")
        wgt = persist.tile([128, NB, NT], f32, tag="wgt")
        comb = persist.tile([128, NB, DH], f32, tag="comb")
        combT = persist.tile([128, NB, 128], bf16, tag="combT")
        mask_sb = persist.tile([128, NT, 2, 128], bf16, tag="masks")
        biasv_sb = persist.tile([128, NT], f32, tag="biasv")
        bb_sb = persist.tile([128, NT], f32, tag="bb")
        Wo_sb = persist.tile([128, D], bf16, tag="Wo")
        wob_sb = persist.tile([128, D], f32, tag="wob")
        ident16 = persist.tile([128, 128], bf16, tag="ident16")
        wq8_sb = persist.tile([128, NT, 4, 2, 128], f8, tag="wq8")
        wk8_sb = persist.tile([128, NT, 4, 2, 128], f8, tag="wk8")
        wv_sb = persist.tile([128, 8, NV], f32r, tag="wv")

        # late-needed constants on the SWDGE queue
        for t in range(NT):
            for kind in range(2):
                nc.gpsimd.dma_start(
                    out=mask_sb[:, t, kind, :], in_=masks[t, kind, :, :]
                )
        nc.gpsimd.dma_start(out=biasv_sb, in_=biasv[:, :])
        nc.gpsimd.dma_start(out=bb_sb, in_=bb[:, :])
        make_identity(nc, ident16)
        nc.vector.memset(V3[:, :, :, DH : DH + 2], 0.0)
        nc.vector.memset(V3[:, :, :, DH : DH + 1], 1.0)

        # RS exchange chunks: one big hidden collective, one small tail one
        RSCH = [(0, 12), (12, 4)]
        rs_in = [
            dram.tile([2 * 128, 64 * nb], bf16, name=f"rs_in{x}")
            for x, (b0, nb) in enumerate(RSCH)
        ]
        rs_out = [
            dram.tile([128, 64 * nb], bf16, name=f"rs_out{x}")
            for x, (b0, nb) in enumerate(RSCH)
        ]

        def proj_slab(c4):
            """QKV projections + router gating for token slab c4 (512 toks)."""
            sl = slice(512 * c4, 512 * (c4 + 1))
            # fp8 x tiles (Q/K proj) first: they unblock the slab's first
            # matmuls, 4 chunks of 2x128 d-rows
            x8s = []
            for c in range(4):
                xc = x8p.tile([128, 2, 512], f8, tag="x8", name=f"x8_{c4}_{c}")
                eng = (nc.sync if c % 2 == 0 else nc.scalar) if c4 else nc.sync
                eng.dma_start(
                    out=xc,
                    in_=x8[256 * c : 256 * (c + 1), sl].rearrange(
                        "(i p) t -> p i t", p=128
                    ),
                )
                x8s.append(xc)
            if c4 == 0:
                # wv follows slab-0's x8 tiles, split per k-chunk so V-proj
                # accumulation starts on k=0 without the whole 1.6MB tensor.
                # Slab 0 queue balance: sync = wq8+x8+wv, scalar = wk8+xs.
                for k in range(8):
                    nc.sync.dma_start(
                        out=wv_sb[:, k, :], in_=wv[128 * k : 128 * (k + 1), :]
                    )
            # f32 x tiles (V proj + router), 8 chunks of 128 d-rows
            xs = []
            for k in range(8):
                xk = xsp.tile([128, 512], f32r, tag="xs", name=f"xs_{c4}_{k}")
                if c4:
                    eng = nc.sync if k % 2 == 0 else nc.scalar
                else:
                    # slab 0 is latency-critical: spill the last chunks to the
                    # otherwise-idle SWDGE queue so the scalar queue drains
                    # sooner
                    eng = nc.scalar if k < 6 else nc.gpsimd
                eng.dma_start(out=xk, in_=xT[128 * k : 128 * (k + 1), sl])
                xs.append(xk)
            if c4 == 1:
                # late-needed output-projection constants, behind slab-1 loads
                nc.gpsimd.dma_start(out=Wo_sb, in_=Wo[:, :])
                nc.gpsimd.dma_start(out=wob_sb, in_=wob[:, :])
            for t in range(NT):
                psq = php.tile([128, 512], f32, tag="php")
                for c in range(4):
                    nc.tensor.matmul(
                        psq, wq8_sb[:, t, c, :, :], x8s[c],
                        start=(c == 0), stop=(c == 3), perf_mode=DR,
                    )
                nc.vector.tensor_copy(qT[t][:, sl], psq)
                psk = php.tile([128, 512], f32, tag="php")
                for c in range(4):
                    nc.tensor.matmul(
                        psk, wk8_sb[:, t, c, :, :], x8s[c],
                        start=(c == 0), stop=(c == 3), perf_mode=DR,
                    )
                nc.vector.tensor_copy(kT[t][:, sl], psk)
            for ib in range(4):
                i = 4 * c4 + ib
                ibo = 128 * ib
                psv = php.tile([128, 512], f32, tag="php")
                for k in range(8):
                    nc.tensor.matmul(
                        psv[:, 0:NV],
                        xs[k][:, ibo : ibo + 128],
                        wv_sb[:, k, :],
                        start=(k == 0),
                        stop=(k == 7),
                    )
                nc.vector.tensor_copy(
                    V3[:, i, :, 0:DH],
                    psv[:, 0 : NT * DH].rearrange("p (n m) -> p n m", n=NT),
                )
                # router gating: top-2-of-3 softmax weights. GPSIMD cannot
                # touch PSUM, so the psv reads stay on DVE; the SBUF-only
                # select+mask+sum runs fused on gpsimd.
                lg = gatep.tile([128, NT], f32, tag="lg")
                nc.vector.tensor_add(lg, psv[:, NT * DH : NT * DH + NT], bb_sb)
                mn2 = gatep.tile([128, 1], f32, tag="mn2")
                nc.vector.tensor_reduce(mn2, lg, axis=AX.X, op=ALU.min)
                eg = gatep.tile([128, NT], f32, tag="eg")
                nc.scalar.activation(eg, lg, ACTF.Exp)
                ew = gatep.tile([128, NT], f32, tag="ew")
                sm = gatep.tile([128, 1], f32, tag="sm")
                nc.vector.scalar_tensor_tensor(
                    ew, lg, mn2, eg, op0=ALU.is_gt, op1=ALU.mult, accum_out=sm
                )
                rc = gatep.tile([128, 1], f32, tag="rc")
                nc.vector.reciprocal(rc, sm)
                nc.vector.tensor_scalar(wgt[:, i, :], ew, rc, None, op0=ALU.mult)

        def attn_chunk(t, c4):
            """S^T -> exp -> AV -> gated combine for type t, query chunk c4."""
            is_causal = t < 2
            has_prev = t >= 1
            base = 4 * c4
            jlo_c = 0 if is_causal else max(0, base - 1)
            es_tiles = {}

            def s_matmul(ps, j, cb, off, hi):
                """S^T matmul + mask/bias identity matmuls into ps[:, cb+off:cb+hi]."""
                has_diag = j >= base
                has_pr = has_prev and base <= j + 1 <= base + 3
                nc.tensor.matmul(
                    ps[:, cb + off : cb + hi],
                    kT[t][:, 128 * j : 128 * (j + 1)],
                    qT[t][:, 512 * c4 + off : 512 * c4 + hi],
                    start=True,
                    stop=not (has_diag or has_pr),
                )
                if has_diag and has_pr:
                    col = cb + 128 * (j - base)
                    nc.tensor.matmul(
                        ps[:, col : col + 256], ident16, mask_sb[:, t, :, :],
                        start=False, stop=True,
                    )
                elif has_diag:
                    col = cb + 128 * (j - base)
                    nc.tensor.matmul(
                        ps[:, col : col + 128], ident16, mask_sb[:, t, 0, :],
                        start=False, stop=True,
                    )
                elif has_pr:
                    col = cb + 128 * (j + 1 - base)
                    nc.tensor.matmul(
                        ps[:, col : col + 128], ident16, mask_sb[:, t, 1, :],
                        start=False, stop=True,
                    )

            # group full-width sub-diagonal j's in pairs sharing one 2-bank
            # psum tile and ONE exp (halves the Act per-instruction overhead
            # on the dominant block population)
            j = jlo_c
            while j < base + 4:
                paired = is_causal and j + 1 < base
                ps = sp.tile([128, 1024], f32, tag="spsum")
                es = expp.tile([128, 1024], bf16, tag="es")
                if paired:
                    s_matmul(ps, j, 0, 0, 512)
                    s_matmul(ps, j + 1, 512, 0, 512)
                    nc.scalar.activation(
                        es, ps, ACTF.Exp,
                        bias=biasv_sb[:, t : t + 1], scale=QKS,
                    )
                    es_tiles[j] = (es, 0)
                    es_tiles[j + 1] = (es, 512)
                    j += 2
                else:
                    off = 128 * max(0, j - base)
                    hi = 512 if is_causal else 128 * min(4, (j - base) + 2)
                    s_matmul(ps, j, 0, off, hi)
                    nc.scalar.activation(
                        es[:, off:hi], ps[:, off:hi], ACTF.Exp,
                        bias=biasv_sb[:, t : t + 1], scale=QKS,
                    )
                    es_tiles[j] = (es, 0)
                    j += 1
            for i in range(base, base + 4):
                jlo_i = 0 if is_causal else max(0, i - 1)
                pav = avp.tile([128, DH + 2], f32, tag="avpsum")
                col = 128 * (i - base)
                for j in range(jlo_i, i + 1):
                    est, cb = es_tiles[j]
                    nc.tensor.matmul(
                        pav,
                        est[:, cb + col : cb + col + 128],
                        V3[:, j, t, :],
                        start=(j == jlo_i),
                        stop=(j == i),
                    )
                rc = avs.tile([128, 1], f32, tag="rcav")
                nc.vector.reciprocal(rc, pav[:, DH : DH + 1])
                if t == 0:
                    nc.vector.tensor_scalar(
                        comb[:, i, :], pav[:, 0:DH], rc, wgt[:, i, t : t + 1],
                        op0=ALU.mult, op1=ALU.mult,
                    )
                else:
                    rcw = avs.tile([128, 1], f32, tag="rcw")
                    nc.vector.tensor_mul(rcw, rc, wgt[:, i, t : t + 1])
                    if t == 1:
                        nc.vector.scalar_tensor_tensor(
                            comb[:, i, :], pav[:, 0:DH], rcw, comb[:, i, :],
                            op0=ALU.mult, op1=ALU.add,
                        )
                    else:
                        cb16 = combp.tile([128, DH], bf16, tag="cb16")
                        nc.vector.scalar_tensor_tensor(
                            cb16, pav[:, 0:DH], rcw, comb[:, i, :],
                            op0=ALU.mult, op1=ALU.add,
                        )
                        # block finished: transpose to [dh, tok] for the exchange
                        tp = avp.tile([128, 128], bf16, tag="avpsum")
                        nc.tensor.transpose(tp, cb16, ident16)
                        nc.scalar.copy(combT[:, i, :], tp)

        def produce_rs(ci):
            b0, nb = RSCH[ci]
            # single full-tensor DMA so the collective has exactly one
            # producing instruction for its input buffer. It waits on the
            # chunk's last attention block, so it must NOT sit on the SP/Act
            # HWDGE queues (head-of-line blocking of x tiles / exp stream) —
            # the idle SWDGE queue is exactly the rs tail chain.
            eng = nc.gpsimd
            rin = eng.dma_start(
                out=rs_in[ci].rearrange("(hh p) (i m) -> p hh i m", hh=2, m=128),
                in_=combT[:, b0 : b0 + nb, :].rearrange(
                    "p (hh i) m -> p hh i m", hh=2
                ),
            )
            rins.append(rin)
            nc.gpsimd.collective_compute(
                "ReduceScatter",
                mybir.AluOpType.add,
                replica_groups=[[0, 1], [2, 3], [4, 5], [6, 7]],
                ins=[rs_in[ci].opt()],
                outs=[rs_out[ci].opt()],
            )

        def consume_rs(ci, row0):
            b0, nb = RSCH[ci]
            hb = nb // 2
            halfT = finp.tile([128, 128 * hb], bf16, tag="halfT", name=f"halfT{ci}")
            # Tile hoists DMA issues toward the front of their queue, and the
            # halfT load waits on its collective — a nosync dep on the last
            # rs_in staging DMA pins it late so its wait can't head-of-line
            # block work that still has to run on this queue.
            from concourse.instruction_name_ordered_set import (
                InstructionNameOrderedSet,
            )
            h = nc.sync.dma_start(out=halfT, in_=rs_out[ci][:, :])
            deps = InstructionNameOrderedSet()
            deps.add(rins[-1].ins.name)
            h.ins.add_nosync_dependencies_from(deps)
            for bi in range(hb):
                for n2 in range(2):
                    nsl = slice(512 * n2, 512 * (n2 + 1))
                    # consumes run after all projections, so the php ring is
                    # free for the Wo psum tiles
                    pf = php.tile([128, 512], f32, tag="php")
                    nc.tensor.matmul(
                        pf,
                        halfT[:, 128 * bi : 128 * (bi + 1)],
                        Wo_sb[:, nsl],
                        start=True,
                        stop=True,
                    )
                    ob = finp.tile([128, 512], f32, tag="ob")
                    nc.vector.tensor_add(ob, pf, wob_sb[:, nsl])
                    r0 = 128 * (row0 + bi)
                    nc.sync.dma_start(out=out[r0 : r0 + 128, nsl], in_=ob)

        # ---------------- main schedule ----------------
        rins = []
        # wq8/wk8 head the two HWDGE queues (slab 0's first consumers)
        nc.sync.dma_start(out=wq8_sb, in_=wq8[:, :, :, :, :])
        nc.scalar.dma_start(out=wk8_sb, in_=wk8[:, :, :, :, :])
        for c4 in range(NCH):
            proj_slab(c4)
            for t in range(NT):
                attn_chunk(t, c4)
            if c4 == 2:
                produce_rs(0)
        produce_rs(1)
        consume_rs(0, row0=0)
        consume_rs(1, row0=6)

    nc.compile()
    return nc


def _prep_inputs(inputs):
    """Build the 8 per-core input maps from the full problem inputs."""
    x = np.asarray(inputs["x"], dtype=np.float32)
    rel_emb = np.asarray(inputs["rel_emb"], dtype=np.float32)
    router_W = np.asarray(inputs["router_W"], dtype=np.float32)
    router_b = np.asarray(inputs["router_b"], dtype=np.float32)
    W_o = np.asarray(inputs["W_o"], dtype=np.float32)
    W_o_b = np.asarray(inputs["W_o_b"], dtype=np.float32)

    # job order is (global, rel, local) = reference type indices (1, 2, 0);
    # permute router columns so logit column t matches job t
    perm = [1, 2, 0]
    router_W = router_W[:, perm]
    router_b = router_b[perm]

    w_by_type = {
        "q": [inputs["global_Wq"], inputs["rel_Wq"], inputs["local_Wq"]],
        "k": [inputs["global_Wk"], inputs["rel_Wk"], inputs["local_Wk"]],
        "v": [inputs["global_Wv"], inputs["rel_Wv"], inputs["local_Wv"]],
    }

    p = np.arange(128)[:, None]
    q = np.arange(128)[None, :]
    tri_causal = np.where(p <= q, 0.0, -BIG).astype(np.float32)      # j<=i
    win_prev = np.where(p >= q, 0.0, -BIG).astype(np.float32)        # j>=i-128

    def relv(h, d):
        return rel_emb[h, np.clip(d, -MRP, MRP) + MRP]

    def pack8(w):
        # [D, DH] -> [128, 4, 2, 128] with d = 256c + 128i + p
        wq = (w * WS).astype(ml_dtypes.float8_e4m3)
        return np.ascontiguousarray(
            wq.reshape(4, 2, 128, DH).transpose(2, 0, 1, 3)
        )

    in_maps = []
    for c in range(8):
        b, h = c // 2, c % 2
        rel0 = float(rel_emb[h, 0])
        m = np.zeros((NT, 2, 128, 128), np.float32)  # cast to bf16 below
        m[0, 0] = tri_causal
        m[1, 0] = MSC * (relv(h, p - q) - rel0) + tri_causal
        m[1, 1] = MSC * (relv(h, p - q - 128) - rel0)
        m[2, 0] = tri_causal
        m[2, 1] = win_prev
        bv = np.zeros((128, NT), np.float32)
        bv[:, 1] = rel0

        wq8_ = np.stack(
            [pack8(np.asarray(w_by_type["q"][t][h], np.float32)) for t in range(NT)]
        ).transpose(1, 0, 2, 3, 4)  # [128, NT, 4, 2, 128]
        wk8_ = np.stack(
            [pack8(np.asarray(w_by_type["k"][t][h], np.float32)) for t in range(NT)]
        ).transpose(1, 0, 2, 3, 4)
        wv_ = np.concatenate(
            [np.asarray(w_by_type["v"][t][h], np.float32) * 0.5 for t in range(NT)]
            + [router_W, np.zeros((D, 1), np.float32)],
            axis=1,
        )
        xb = np.ascontiguousarray(x[b].T)
        in_maps.append(
            {
                "xT": xb,
                "x8": xb.astype(ml_dtypes.float8_e4m3),
                "wq8": np.ascontiguousarray(wq8_),
                "wk8": np.ascontiguousarray(wk8_),
                "wv": np.ascontiguousarray(wv_),
                "masks": m.astype(ml_dtypes.bfloat16),
                "biasv": bv,
                "bb": np.broadcast_to(router_b, (128, NT)).copy(),
                "Wo": np.ascontiguousarray(W_o).astype(ml_dtypes.bfloat16),
                "wob": np.broadcast_to(W_o_b, (128, D)).copy(),
            }
        )
    return in_maps


def kernel(**inputs) -> np.ndarray:
    from concourse.bass_utils import run_bass_kernel_spmd

    if "nc" not in _CACHE:
        _CACHE["nc"] = _build_nc()
    nc = _CACHE["nc"]

    in_maps = _prep_inputs(inputs)
    trace = os.environ.get("KERNEL_TRACE", "0") == "1"
    res = run_bass_kernel_spmd(
        nc, in_maps, core_ids=list(range(8)), trace=trace
    )
    _CACHE["last_result"] = res

    out = np.empty((B, T, D), np.float32)
    RSCH = [(0, 12), (12, 4)]
    for c in range(8):
        b, h = c // 2, c % 2
        r = res.results[c]["out"]
        row0 = 0
        for b0, nb in RSCH:
            hb = nb // 2
            t0 = 128 * (b0 + h * hb)
            out[b, t0 : t0 + 128 * hb] = r[128 * row0 : 128 * (row0 + hb)]
            row0 += hb
    return out


# revision 51
# speedup vs baseline: 1.0186x; 1.0186x over previous
"""Trainium2 Bass kernel for nn_MixtureOfAttentionHeads.

Sharding: 8 cores = 4 batches x 2 heads. Core c handles batch c//2, head c%2:
it computes all three attention types (global/rel/local) for its head over the
full sequence, applies the router gate per token, then a per-chunk pairwise
ReduceScatter sums the two heads (0.5 factor folded into Wv) and splits each
512-token chunk across the pair; each core projects its quarter-chunks with
W_o. The four chunk collectives are pipelined against later-chunk compute;
chunk k's RS output is consumed (W_o + store) while chunk k+1 computes.

Q/K projections run as fp8(e4m3) DoubleRow matmuls (weights prescaled x32,
unscaled in the exp), which quarters their PE cost; V/router projection stays
f32r for router-logit fidelity (top-k flips dominate the error budget).

The SPMD program is identical on every core; all per-core differences
(batch, head weights, rel-bias tiles) arrive as input data.
"""

import os
import sys

sys.path.insert(0, "/opt/trn_rl_repo")

import numpy as np
import ml_dtypes

# --- problem constants (hardcoded per contract) ---
B, T, D, DH = 4, 2048, 1024, 128
H, NT, TOPK = 2, 3, 2
WIN, MRP = 128, 32
SCALE = float(np.sqrt(DH))
NB = T // 128          # 16 query/key blocks
NCH = T // 512         # 4 query chunks of 512
BIG = 1.0e9
WS = 32.0              # fp8 weight prescale (q,k each carry x32)
QKS = 1.0 / (WS * WS * SCALE)   # exp scale undoing both prescales
MSC = WS * WS * SCALE  # host-side premultiplier for additive score masks

_CACHE = {}


def _build_nc():
    import concourse.bass as bass  # noqa: F401
    import concourse.mybir as mybir
    import concourse.tile as tile
    from concourse import bacc
    from concourse.masks import make_identity
    from contextlib import ExitStack

    dt = mybir.dt
    f32, f32r, bf16, f8 = dt.float32, dt.float32r, dt.bfloat16, dt.float8e4
    AX = mybir.AxisListType
    ALU = mybir.AluOpType
    ACTF = mybir.ActivationFunctionType
    DR = mybir.MatmulPerfMode.DoubleRow

    nc = bacc.Bacc("TRN2", target_bir_lowering=False, num_devices=8)

    NV = NT * DH + NT + 1  # 388 (even, fp32r requirement)

    xT = nc.dram_tensor("xT", [D, T], f32r, kind="ExternalInput")
    x8 = nc.dram_tensor("x8", [D, T], f8, kind="ExternalInput")
    wq8 = nc.dram_tensor("wq8", [128, NT, 4, 2, 128], f8, kind="ExternalInput")
    wk8 = nc.dram_tensor("wk8", [128, NT, 4, 2, 128], f8, kind="ExternalInput")
    wv = nc.dram_tensor("wv", [D, NV], f32r, kind="ExternalInput")
    masks = nc.dram_tensor("masks", [NT, 2, 128, 128], bf16, kind="ExternalInput")
    biasv = nc.dram_tensor("biasv", [128, NT], f32, kind="ExternalInput")
    bb = nc.dram_tensor("bb", [128, NT], f32, kind="ExternalInput")
    Wo = nc.dram_tensor("Wo", [DH, D], bf16, kind="ExternalInput")
    wob = nc.dram_tensor("wob", [128, D], f32, kind="ExternalInput")
    out = nc.dram_tensor("out", [T // 2, D], f32, kind="ExternalOutput")

    with tile.TileContext(nc) as tc, ExitStack() as ctx:
        persist = ctx.enter_context(tc.tile_pool(name="persist", bufs=1))
        xsp = ctx.enter_context(tc.tile_pool(name="xsp", bufs=16))
        x8p = ctx.enter_context(tc.tile_pool(name="x8p", bufs=8))
        gatep = ctx.enter_context(tc.tile_pool(name="gatep", bufs=8))
        expp = ctx.enter_context(tc.tile_pool(name="expp", bufs=18))
        avs = ctx.enter_context(tc.tile_pool(name="avs", bufs=8))
        combp = ctx.enter_context(tc.tile_pool(name="combp", bufs=4))
        finp = ctx.enter_context(tc.tile_pool(name="finp", bufs=4))
        dram = ctx.enter_context(tc.tile_pool(name="dram", bufs=1, space="DRAM"))
        php = ctx.enter_context(tc.tile_pool(name="php", bufs=2, space="PSUM"))
        sp = ctx.enter_context(tc.tile_pool(name="sp", bufs=2, space="PSUM"))
        avp = ctx.enter_context(tc.tile_pool(name="avp", bufs=2, space="PSUM"))

        # ---- persistent SBUF tensors ----
        qT = [persist.tile([128, T], bf16, tag=f"qT{t}", name=f"qT{t}") for t in range(NT)]
        kT = [persist.tile([128, T], bf16, tag=f"kT{t}", name=f"kT{t}") for t in range(NT)]
        V3 = persist.tile([128, NB, NT, DH + 2], bf16, tag="V3")
        wgt = persist.tile([128, NB, NT], f32, tag="wgt")
        comb = persist.tile([128, NB, DH], f32, tag="comb")
        combT = persist.tile([128, NB, 128], bf16, tag="combT")
        mask_sb = persist.tile([128, NT, 2, 128], bf16, tag="masks")
        biasv_sb = persist.tile([128, NT], f32, tag="biasv")
        bb_sb = persist.tile([128, NT], f32, tag="bb")
        Wo_sb = persist.tile([128, D], bf16, tag="Wo")
        wob_sb = persist.tile([128, D], f32, tag="wob")
        ident16 = persist.tile([128, 128], bf16, tag="ident16")
        wq8_sb = persist.tile([128, NT, 4, 2, 128], f8, tag="wq8")
        wk8_sb = persist.tile([128, NT, 4, 2, 128], f8, tag="wk8")
        wv_sb = persist.tile([128, 8, NV], f32r, tag="wv")

        # late-needed constants on the SWDGE queue
        for t in range(NT):
            for kind in range(2):
                nc.gpsimd.dma_start(
                    out=mask_sb[:, t, kind, :], in_=masks[t, kind, :, :]
                )
        nc.gpsimd.dma_start(out=biasv_sb, in_=biasv[:, :])
        nc.gpsimd.dma_start(out=bb_sb, in_=bb[:, :])
        make_identity(nc, ident16)
        nc.vector.memset(V3[:, :, :, DH : DH + 2], 0.0)
        nc.vector.memset(V3[:, :, :, DH : DH + 1], 1.0)

        # RS exchange chunks: one big hidden collective, one small tail one
        RSCH = [(0, 12), (12, 4)]
        rs_in = [
            dram.tile([2 * 128, 64 * nb], bf16, name=f"rs_in{x}")
            for x, (b0, nb) in enumerate(RSCH)
        ]
        rs_out = [
            dram.tile([128, 64 * nb], bf16, name=f"rs_out{x}")
            for x, (b0, nb) in enumerate(RSCH)
        ]

        def proj_slab(c4):
            """QKV projections + router gating for token slab c4 (512 toks)."""
            sl = slice(512 * c4, 512 * (c4 + 1))
            # fp8 x tiles (Q/K proj) first: they unblock the slab's first
            # matmuls, 4 chunks of 2x128 d-rows
            x8s = []
            for c in range(4):
                xc = x8p.tile([128, 2, 512], f8, tag="x8", name=f"x8_{c4}_{c}")
                eng = (nc.sync if c % 2 == 0 else nc.scalar) if c4 else nc.sync
                eng.dma_start(
                    out=xc,
                    in_=x8[256 * c : 256 * (c + 1), sl].rearrange(
                        "(i p) t -> p i t", p=128
                    ),
                )
                x8s.append(xc)
            if c4 == 0:
                # wv follows slab-0's x8 tiles, split per k-chunk so V-proj
                # accumulation starts on k=0 without the whole 1.6MB tensor.
                # Slab 0 queue balance: sync = wq8+x8+wv, scalar = wk8+xs.
                for k in range(8):
                    nc.sync.dma_start(
                        out=wv_sb[:, k, :], in_=wv[128 * k : 128 * (k + 1), :]
                    )
            # f32 x tiles (V proj + router), 8 chunks of 128 d-rows
            xs = []
            for k in range(8):
                xk = xsp.tile([128, 512], f32r, tag="xs", name=f"xs_{c4}_{k}")
                if c4:
                    eng = nc.sync if k % 2 == 0 else nc.scalar
                else:
                    # slab 0 is latency-critical: spill the last chunks to the
                    # otherwise-idle SWDGE queue so the scalar queue drains
                    # sooner
                    eng = nc.scalar if k < 6 else nc.gpsimd
                eng.dma_start(out=xk, in_=xT[128 * k : 128 * (k + 1), sl])
                xs.append(xk)
            if c4 == 1:
                # late-needed output-projection constants, behind slab-1 loads
                nc.gpsimd.dma_start(out=Wo_sb, in_=Wo[:, :])
                nc.gpsimd.dma_start(out=wob_sb, in_=wob[:, :])
            for t in range(NT):
                psq = php.tile([128, 512], f32, tag="php")
                for c in range(4):
                    nc.tensor.matmul(
                        psq, wq8_sb[:, t, c, :, :], x8s[c],
                        start=(c == 0), stop=(c == 3), perf_mode=DR,
                    )
                nc.vector.tensor_copy(qT[t][:, sl], psq)
                psk = php.tile([128, 512], f32, tag="php")
                for c in range(4):
                    nc.tensor.matmul(
                        psk, wk8_sb[:, t, c, :, :], x8s[c],
                        start=(c == 0), stop=(c == 3), perf_mode=DR,
                    )
                nc.vector.tensor_copy(kT[t][:, sl], psk)
            for ib in range(4):
                i = 4 * c4 + ib
                ibo = 128 * ib
                psv = php.tile([128, 512], f32, tag="php")
                for k in range(8):
                    nc.tensor.matmul(
                        psv[:, 0:NV],
                        xs[k][:, ibo : ibo + 128],
                        wv_sb[:, k, :],
                        start=(k == 0),
                        stop=(k == 7),
                    )
                nc.vector.tensor_copy(
                    V3[:, i, :, 0:DH],
                    psv[:, 0 : NT * DH].rearrange("p (n m) -> p n m", n=NT),
                )
                # router gating: top-2-of-3 softmax weights. GPSIMD cannot
                # touch PSUM, so the psv reads stay on DVE; the SBUF-only
                # select+mask+sum runs fused on gpsimd.
                lg = gatep.tile([128, NT], f32, tag="lg")
                nc.vector.tensor_add(lg, psv[:, NT * DH : NT * DH + NT], bb_sb)
                mn2 = gatep.tile([128, 1], f32, tag="mn2")
                nc.vector.tensor_reduce(mn2, lg, axis=AX.X, op=ALU.min)
                eg = gatep.tile([128, NT], f32, tag="eg")
                nc.scalar.activation(eg, lg, ACTF.Exp)
                ew = gatep.tile([128, NT], f32, tag="ew")
                sm = gatep.tile([128, 1], f32, tag="sm")
                nc.vector.scalar_tensor_tensor(
                    ew, lg, mn2, eg, op0=ALU.is_gt, op1=ALU.mult, accum_out=sm
                )
                rc = gatep.tile([128, 1], f32, tag="rc")
                nc.vector.reciprocal(rc, sm)
                nc.vector.tensor_scalar(wgt[:, i, :], ew, rc, None, op0=ALU.mult)

        def attn_chunk(t, c4):
            """S^T -> exp -> AV -> gated combine for type t, query chunk c4."""
            is_causal = t < 2
            has_prev = t >= 1
            base = 4 * c4
            jlo_c = 0 if is_causal else max(0, base - 1)
            es_tiles = {}

            def s_matmul(ps, j, cb, off, hi):
                """S^T matmul + mask/bias identity matmuls into ps[:, cb+off:cb+hi]."""
                has_diag = j >= base
                has_pr = has_prev and base <= j + 1 <= base + 3
                nc.tensor.matmul(
                    ps[:, cb + off : cb + hi],
                    kT[t][:, 128 * j : 128 * (j + 1)],
                    qT[t][:, 512 * c4 + off : 512 * c4 + hi],
                    start=True,
                    stop=not (has_diag or has_pr),
                )
                if has_diag and has_pr:
                    col = cb + 128 * (j - base)
                    nc.tensor.matmul(
                        ps[:, col : col + 256], ident16, mask_sb[:, t, :, :],
                        start=False, stop=True,
                    )
                elif has_diag:
                    col = cb + 128 * (j - base)
                    nc.tensor.matmul(
                        ps[:, col : col + 128], ident16, mask_sb[:, t, 0, :],
                        start=False, stop=True,
                    )
                elif has_pr:
                    col = cb + 128 * (j + 1 - base)
                    nc.tensor.matmul(
                        ps[:, col : col + 128], ident16, mask_sb[:, t, 1, :],
                        start=False, stop=True,
                    )

            # two j-blocks share each 2-bank psum tile; a pair of full-width
            # sub-diagonal blocks gets ONE exp (halves the Act per-instruction
            # overhead on the dominant block population), partial-width
            # blocks get per-slice exps
            def wid(j):
                off = 128 * max(0, j - base)
                hi = 512 if is_causal else 128 * min(4, (j - base) + 2)
                return off, hi

            j = jlo_c
            while j < base + 4:
                ps = sp.tile([128, 1024], f32, tag="spsum")
                es = expp.tile([128, 1024], bf16, tag="es")
                pair = j + 1 < base + 4
                if pair:
                    o0, h0 = wid(j)
                    o1, h1 = wid(j + 1)
                    s_matmul(ps, j, 0, o0, h0)
                    s_matmul(ps, j + 1, 512, o1, h1)
                    if (o0, h0) == (0, 512) and (o1, h1) == (0, 512):
                        nc.scalar.activation(
                            es, ps, ACTF.Exp,
                            bias=biasv_sb[:, t : t + 1], scale=QKS,
                        )
                    else:
                        nc.scalar.activation(
                            es[:, o0:h0], ps[:, o0:h0], ACTF.Exp,
                            bias=biasv_sb[:, t : t + 1], scale=QKS,
                        )
                        nc.scalar.activation(
                            es[:, 512 + o1 : 512 + h1], ps[:, 512 + o1 : 512 + h1],
                            ACTF.Exp,
                            bias=biasv_sb[:, t : t + 1], scale=QKS,
                        )
                    es_tiles[j] = (es, 0)
                    es_tiles[j + 1] = (es, 512)
                    j += 2
                else:
                    off, hi = wid(j)
                    s_matmul(ps, j, 0, off, hi)
                    nc.scalar.activation(
                        es[:, off:hi], ps[:, off:hi], ACTF.Exp,
                        bias=biasv_sb[:, t : t + 1], scale=QKS,
                    )
                    es_tiles[j] = (es, 0)
                    j += 1
            for i in range(base, base + 4):
                jlo_i = 0 if is_causal else max(0, i - 1)
                pav = avp.tile([128, DH + 2], f32, tag="avpsum")
                col = 128 * (i - base)
                for j in range(jlo_i, i + 1):
                    est, cb = es_tiles[j]
                    nc.tensor.matmul(
                        pav,
                        est[:, cb + col : cb + col + 128],
                        V3[:, j, t, :],
                        start=(j == jlo_i),
                        stop=(j == i),
                    )
                rc = avs.tile([128, 1], f32, tag="rcav")
                nc.vector.reciprocal(rc, pav[:, DH : DH + 1])
                if t == 0:
                    nc.vector.tensor_scalar(
                        comb[:, i, :], pav[:, 0:DH], rc, wgt[:, i, t : t + 1],
                        op0=ALU.mult, op1=ALU.mult,
                    )
                else:
                    rcw = avs.tile([128, 1], f32, tag="rcw")
                    nc.vector.tensor_mul(rcw, rc, wgt[:, i, t : t + 1])
                    if t == 1:
                        nc.vector.scalar_tensor_tensor(
                            comb[:, i, :], pav[:, 0:DH], rcw, comb[:, i, :],
                            op0=ALU.mult, op1=ALU.add,
                        )
                    else:
                        cb16 = combp.tile([128, DH], bf16, tag="cb16")
                        nc.vector.scalar_tensor_tensor(
                            cb16, pav[:, 0:DH], rcw, comb[:, i, :],
                            op0=ALU.mult, op1=ALU.add,
                        )
                        # block finished: transpose to [dh, tok] for the exchange
                        tp = avp.tile([128, 128], bf16, tag="avpsum")
                        nc.tensor.transpose(tp, cb16, ident16)
                        nc.scalar.copy(combT[:, i, :], tp)

        def produce_rs(ci):
            b0, nb = RSCH[ci]
            # single full-tensor DMA so the collective has exactly one
            # producing instruction for its input buffer. It waits on the
            # chunk's last attention block, so it must NOT sit on the SP/Act
            # HWDGE queues (head-of-line blocking of x tiles / exp stream) —
            # the idle SWDGE queue is exactly the rs tail chain.
            eng = nc.gpsimd
            rin = eng.dma_start(
                out=rs_in[ci].rearrange("(hh p) (i m) -> p hh i m", hh=2, m=128),
                in_=combT[:, b0 : b0 + nb, :].rearrange(
                    "p (hh i) m -> p hh i m", hh=2
                ),
            )
            rins.append(rin)
            nc.gpsimd.collective_compute(
                "ReduceScatter",
                mybir.AluOpType.add,
                replica_groups=[[0, 1], [2, 3], [4, 5], [6, 7]],
                ins=[rs_in[ci].opt()],
                outs=[rs_out[ci].opt()],
            )

        def consume_rs(ci, row0):
            b0, nb = RSCH[ci]
            hb = nb // 2
            halfT = finp.tile([128, 128 * hb], bf16, tag="halfT", name=f"halfT{ci}")
            # Tile hoists DMA issues toward the front of their queue, and the
            # halfT load waits on its collective — a nosync dep on the last
            # rs_in staging DMA pins it late so its wait can't head-of-line
            # block work that still has to run on this queue.
            from concourse.instruction_name_ordered_set import (
                InstructionNameOrderedSet,
            )
            h = nc.sync.dma_start(out=halfT, in_=rs_out[ci][:, :])
            deps = InstructionNameOrderedSet()
            deps.add(rins[-1].ins.name)
            h.ins.add_nosync_dependencies_from(deps)
            for bi in range(hb):
                for n2 in range(2):
                    nsl = slice(512 * n2, 512 * (n2 + 1))
                    # consumes run after all projections, so the php ring is
                    # free for the Wo psum tiles
                    pf = php.tile([128, 512], f32, tag="php")
                    nc.tensor.matmul(
                        pf,
                        halfT[:, 128 * bi : 128 * (bi + 1)],
                        Wo_sb[:, nsl],
                        start=True,
                        stop=True,
                    )
                    ob = finp.tile([128, 512], f32, tag="ob")
                    nc.vector.tensor_add(ob, pf, wob_sb[:, nsl])
                    r0 = 128 * (row0 + bi)
                    nc.sync.dma_start(out=out[r0 : r0 + 128, nsl], in_=ob)

        # ---------------- main schedule ----------------
        rins = []
        # wq8/wk8 head the two HWDGE queues (slab 0's first consumers)
        nc.sync.dma_start(out=wq8_sb, in_=wq8[:, :, :, :, :])
        nc.scalar.dma_start(out=wk8_sb, in_=wk8[:, :, :, :, :])
        for c4 in range(NCH):
            proj_slab(c4)
            for t in range(NT):
                attn_chunk(t, c4)
            if c4 == 2:
                produce_rs(0)
        produce_rs(1)
        consume_rs(0, row0=0)
        consume_rs(1, row0=6)

    nc.compile()
    return nc


def _prep_inputs(inputs):
    """Build the 8 per-core input maps from the full problem inputs."""
    x = np.asarray(inputs["x"], dtype=np.float32)
    rel_emb = np.asarray(inputs["rel_emb"], dtype=np.float32)
    router_W = np.asarray(inputs["router_W"], dtype=np.float32)
    router_b = np.asarray(inputs["router_b"], dtype=np.float32)
    W_o = np.asarray(inputs["W_o"], dtype=np.float32)
    W_o_b = np.asarray(inputs["W_o_b"], dtype=np.float32)

    # job order is (global, rel, local) = reference type indices (1, 2, 0);
    # permute router columns so logit column t matches job t
    perm = [1, 2, 0]
    router_W = router_W[:, perm]
    router_b = router_b[perm]

    w_by_type = {
        "q": [inputs["global_Wq"], inputs["rel_Wq"], inputs["local_Wq"]],
        "k": [inputs["global_Wk"], inputs["rel_Wk"], inputs["local_Wk"]],
        "v": [inputs["global_Wv"], inputs["rel_Wv"], inputs["local_Wv"]],
    }

    p = np.arange(128)[:, None]
    q = np.arange(128)[None, :]
    tri_causal = np.where(p <= q, 0.0, -BIG).astype(np.float32)      # j<=i
    win_prev = np.where(p >= q, 0.0, -BIG).astype(np.float32)        # j>=i-128

    def relv(h, d):
        return rel_emb[h, np.clip(d, -MRP, MRP) + MRP]

    def pack8(w):
        # [D, DH] -> [128, 4, 2, 128] with d = 256c + 128i + p
        wq = (w * WS).astype(ml_dtypes.float8_e4m3)
        return np.ascontiguousarray(
            wq.reshape(4, 2, 128, DH).transpose(2, 0, 1, 3)
        )

    in_maps = []
    for c in range(8):
        b, h = c // 2, c % 2
        rel0 = float(rel_emb[h, 0])
        m = np.zeros((NT, 2, 128, 128), np.float32)  # cast to bf16 below
        m[0, 0] = tri_causal
        m[1, 0] = MSC * (relv(h, p - q) - rel0) + tri_causal
        m[1, 1] = MSC * (relv(h, p - q - 128) - rel0)
        m[2, 0] = tri_causal
        m[2, 1] = win_prev
        bv = np.zeros((128, NT), np.float32)
        bv[:, 1] = rel0

        wq8_ = np.stack(
            [pack8(np.asarray(w_by_type["q"][t][h], np.float32)) for t in range(NT)]
        ).transpose(1, 0, 2, 3, 4)  # [128, NT, 4, 2, 128]
        wk8_ = np.stack(
            [pack8(np.asarray(w_by_type["k"][t][h], np.float32)) for t in range(NT)]
        ).transpose(1, 0, 2, 3, 4)
        wv_ = np.concatenate(
            [np.asarray(w_by_type["v"][t][h], np.float32) * 0.5 for t in range(NT)]
            + [router_W, np.zeros((D, 1), np.float32)],
            axis=1,
        )
        xb = np.ascontiguousarray(x[b].T)
        in_maps.append(
            {
                "xT": xb,
                "x8": xb.astype(ml_dtypes.float8_e4m3),
                "wq8": np.ascontiguousarray(wq8_),
                "wk8": np.ascontiguousarray(wk8_),
                "wv": np.ascontiguousarray(wv_),
                "masks": m.astype(ml_dtypes.bfloat16),
                "biasv": bv,
                "bb": np.broadcast_to(router_b, (128, NT)).copy(),
                "Wo": np.ascontiguousarray(W_o).astype(ml_dtypes.bfloat16),
                "wob": np.broadcast_to(W_o_b, (128, D)).copy(),
            }
        )
    return in_maps


def kernel(**inputs) -> np.ndarray:
    from concourse.bass_utils import run_bass_kernel_spmd

    if "nc" not in _CACHE:
        _CACHE["nc"] = _build_nc()
    nc = _CACHE["nc"]

    in_maps = _prep_inputs(inputs)
    trace = os.environ.get("KERNEL_TRACE", "0") == "1"
    res = run_bass_kernel_spmd(
        nc, in_maps, core_ids=list(range(8)), trace=trace
    )
    _CACHE["last_result"] = res

    out = np.empty((B, T, D), np.float32)
    RSCH = [(0, 12), (12, 4)]
    for c in range(8):
        b, h = c // 2, c % 2
        r = res.results[c]["out"]
        row0 = 0
        for b0, nb in RSCH:
            hb = nb // 2
            t0 = 128 * (b0 + h * hb)
            out[b, t0 : t0 + 128 * hb] = r[128 * row0 : 128 * (row0 + hb)]
            row0 += hb
    return out
